# revision 43
# baseline (speedup 1.0000x reference)
"""Trainium2 Bass kernel for nn_BottleneckFusion (STCN memory readout + ResBlock
+ CBAM + PSP + bottleneck), 8-core SPMD.

Sharding: core c -> (batch b = c//2, half h = c%2).
  Phase A (attention): TM split across the pair (4 memory frames each);
    flash-style combine of (unnormalized value, sumexp) via pairwise AllGather,
    trimmed to the 18 query rows each receiver actually needs (576 cols).
  Phase B: each core computes xb ONLY for its own 16 rows (no halo recompute);
    the pair exchanges own-row xb with one AllGather whose two slots land
    statically (slot0 = image rows 0..15, slot1 = rows 16..31). Everything
    after (CBAM gate, spatial gate, fused, PSP pools) is computed locally and
    redundantly on the full image; the bottleneck conv emits own rows only.

kernel(**inputs) takes the FULL unsharded inputs and returns the FULL output.
"""
import sys

sys.path.insert(0, "/opt/trn_rl_repo")

import numpy as np
import ml_dtypes

import concourse.bass as bass
import concourse.bacc as bacc
import concourse.mybir as mybir
import concourse.tile as tile
from concourse.bass_utils import run_bass_kernel_spmd

BF16 = ml_dtypes.bfloat16
F16 = np.float16
bf = mybir.dt.bfloat16
f16 = mybir.dt.float16
f32 = mybir.dt.float32
AF = mybir.ActivationFunctionType
ALU = mybir.AluOpType
AX = mybir.AxisListType

N_CORES = 8
B, TM, CIN, CK, CV, COUT, H, W = 4, 8, 256, 64, 256, 256, 32, 32
EPS = 1e-5

XR = 20                    # x window rows (image r0-2 .. r0+17)
RR = 18                    # r1 rows (image r0-1 .. r0+16)
VCOLS = 576                # 18 exchanged val rows * 32
PAIRS = [[0, 1], [2, 3], [4, 5], [6, 7]]
UPS = (2, 4, 8)            # upsampled PSP scales
# pool pyramid offsets within the 85-entry full pyramid [s1|s2|s4|s8]
POFF = {1: 0, 2: 1, 4: 5, 8: 21}
PDOFF = {8: 0, 1: 64, 4: 65, 2: 81}
SI = {1: 0, 2: 1, 4: 2, 8: 3}


def interp_matrix(s_in, s_out=32):
    if s_in == 1:
        return np.ones((s_out, 1), np.float32)
    c = np.arange(s_out) * (s_in - 1) / (s_out - 1)
    lo = np.floor(c).astype(np.int64)
    hi = np.minimum(lo + 1, s_in - 1)
    w = (c - lo).astype(np.float32)
    M = np.zeros((s_out, s_in), np.float32)
    M[np.arange(s_out), lo] += 1.0 - w
    M[np.arange(s_out), hi] += w
    return M


# ---------------------------------------------------------------------------
# Host-side input preparation
# ---------------------------------------------------------------------------

def _pad_hw(a):
    out = np.zeros(a.shape[:-2] + (34, 34), a.dtype)
    out[..., 1:33, 1:33] = a
    return out


def _chw_chunks(a):
    """[256, ...] -> [128, 2, ...] (partition, chunk)."""
    return a.reshape(2, 128, *a.shape[1:]).transpose(
        1, 0, *range(2, a.ndim + 1))


def prep_core_inputs(inputs, core):
    b, h = core // 2, core % 2
    r0 = 16 * h
    g = {}

    f16_q = np.asarray(inputs["f16_q"], np.float32)
    f16_m = np.asarray(inputs["f16_m"], np.float32)
    value_m = np.asarray(inputs["value_m"], np.float32)

    # xm: [128, 2, 4, 34, 34] padded memory frames
    src = f16_m[b, 4 * h: 4 * h + 4]                        # [4, 256, 32, 32]
    src = src.reshape(4, 2, 128, 32, 32).transpose(2, 1, 0, 3, 4)
    g["xm"] = _pad_hw(src).astype(F16)

    # xq: [128, 2, 34, 34] padded query
    q = _chw_chunks(f16_q[b, 0])                            # [128, 2, 32, 32]
    g["xq"] = _pad_hw(q).astype(F16)

    # vT: [128, 32, 256] transposed value
    V = value_m[b][:, 4 * h: 4 * h + 4].reshape(CV, 4096)
    g["vT"] = np.ascontiguousarray(
        V.T.reshape(32, 128, CV).transpose(1, 0, 2)).astype(BF16)

    # x window q-part: [128, 2, XR, 34], row l = image r0-2+l
    qw = np.zeros((128, 2, XR, 34), np.float32)
    for l in range(XR):
        img = r0 - 2 + l
        if 0 <= img <= 31:
            qw[:, :, l, 1:33] = q[:, :, img, :]
    g["xqb_raw"] = qw.astype(F16)
    g["xqb_relu"] = np.maximum(qw, 0.0).astype(F16)

    pk_w = np.asarray(inputs["pk_w"], np.float32)
    g["pk_wT"] = np.ascontiguousarray(
        pk_w.reshape(CK, 2, 128, 3, 3).transpose(2, 1, 3, 4, 0)).astype(F16)
    pk_b = np.asarray(inputs["pk_b"], np.float32)
    g["pkb2"] = np.concatenate([pk_b, pk_b]).reshape(128, 1).astype(np.float32)

    def conv_lhsT(w, kc):
        co = w.shape[0]
        return np.ascontiguousarray(
            w.reshape(co, kc, 128, 3, 3).transpose(2, 1, 3, 4, 0)).astype(F16)

    g["rb1_wT"] = conv_lhsT(np.asarray(inputs["rb1_w"], np.float32), 4)
    g["rb2_wT"] = conv_lhsT(np.asarray(inputs["rb2_w"], np.float32), 2)
    g["rbd_wT"] = conv_lhsT(np.asarray(inputs["rbd_w"], np.float32), 4)
    g["rb1_b"] = np.asarray(inputs["rb1_b"], np.float32).reshape(2, 128).T.copy()
    g["xb_bias"] = (np.asarray(inputs["rb2_b"], np.float32)
                    + np.asarray(inputs["rbd_b"], np.float32)
                    ).reshape(2, 128).T.copy()

    w1 = np.asarray(inputs["mlp_w1"], np.float32)           # [16, 256]
    g["mlp_w1T"] = np.ascontiguousarray(
        w1.reshape(16, 2, 128).transpose(2, 1, 0)).copy()   # [128, 2, 16]
    g["mlp_b1"] = np.asarray(inputs["mlp_b1"], np.float32).reshape(16, 1).copy()
    g["mlp_w2T"] = np.ascontiguousarray(
        np.asarray(inputs["mlp_w2"], np.float32).T).copy()  # [16, 256]
    g["mlp_b2x2"] = (2.0 * np.asarray(inputs["mlp_b2"], np.float32)
                     ).reshape(2, 128).T.copy()

    spw = np.asarray(inputs["sp_w"], np.float32)[0]       # [2, 7, 7]
    g["spw_r"] = np.ascontiguousarray(
        spw.reshape(14, 7)).astype(np.float16)                # [(ch,dy), dx]
    bn_scale = float(np.asarray(inputs["sp_g"], np.float32)[0]) / float(
        np.sqrt(1.0 + EPS))
    bn_bias = float(np.asarray(inputs["sp_b"], np.float32)[0])
    g["bn_sb"] = np.array([[bn_scale, bn_bias]], np.float32)

    pw = np.zeros((128, 2, 4, 64), np.float32)
    for si, s in enumerate((1, 2, 4, 8)):
        wc = np.asarray(inputs[f"psp_w{s}"], np.float32)[:, :, 0, 0]
        scale = 16.0 / ((32 // s) ** 2)
        pw[:, :, si, :] = (wc.T * scale).reshape(2, 128, 64).transpose(1, 0, 2)
    g["psp_wT"] = pw.astype(F16)

    # folded upsample operators: Wup[k=(jr*s+jc), si, (r*32+c)] =
    # M[r0+r, jr] * M[c, jc]
    Wup = np.zeros((64, 3, 512), np.float32)
    for si, s in enumerate(UPS):
        M = interp_matrix(s)
        Mrr = M[r0: r0 + 16, :]                 # [16, s]
        for jr in range(s):
            for jc in range(s):
                Wup[jr * s + jc, si, :] = np.outer(Mrr[:, jr],
                                                   M[:, jc]).reshape(512)
    g["Wup"] = Wup.astype(BF16)

    bott_w = np.asarray(inputs["bott_w"], np.float32)[:, :, 0, 0]
    g["bott_wT"] = np.ascontiguousarray(
        bott_w.reshape(COUT, 4, 128).transpose(2, 1, 0)).astype(F16)
    g["bott_b"] = np.asarray(inputs["bott_b"], np.float32).reshape(2, 128).T.copy()

    # r1 row validity: r1 row t = image r0-1+t
    rmask = np.zeros((1, RR, 34), np.float16)
    for t in range(RR):
        if 0 <= r0 - 1 + t <= 31:
            rmask[0, t, :] = 1.0
    g["rmask"] = rmask

    # slot selection for the trimmed val AG: slot s holds my window iff s != h
    sm = np.zeros((128, 2), np.float32)
    sm[:, 1 - h] = 1.0
    g["slotmask"] = sm

    sel = np.zeros((2, 256), np.float16)
    sel[0, 0:128] = 1.0
    sel[1, 128:256] = 1.0
    g["sel"] = sel

    g["ident"] = np.eye(128, dtype=np.float32)
    g["ident16"] = np.eye(128, dtype=np.float16)
    return g


INPUT_SPECS = [
    ("xm", [128, 2, 4, 34, 34], f16),
    ("xq", [128, 2, 34, 34], f16),
    ("vT", [128, 32, 256], bf),
    ("xqb_raw", [128, 2, XR, 34], f16),
    ("xqb_relu", [128, 2, XR, 34], f16),
    ("pk_wT", [128, 2, 3, 3, 64], f16),
    ("pkb2", [128, 1], f32),
    ("rb1_wT", [128, 4, 3, 3, 256], f16),
    ("rb2_wT", [128, 2, 3, 3, 256], f16),
    ("rbd_wT", [128, 4, 3, 3, 256], f16),
    ("rb1_b", [128, 2], f32),
    ("xb_bias", [128, 2], f32),
    ("mlp_w1T", [128, 2, 16], f32),
    ("mlp_b1", [16, 1], f32),
    ("mlp_w2T", [16, 256], f32),
    ("mlp_b2x2", [128, 2], f32),
    ("spw_r", [14, 7], f16),
    ("bn_sb", [1, 2], f32),
    ("psp_wT", [128, 2, 4, 64], f16),
    ("Wup", [64, 3, 512], bf),
    ("bott_wT", [128, 4, 256], f16),
    ("bott_b", [128, 2], f32),
    ("sel", [2, 256], f16),
    ("ident", [128, 128], f32),
    ("ident16", [128, 128], f16),
    ("rmask", [1, RR, 34], f16),
    ("slotmask", [128, 2], f32),
]


# ---------------------------------------------------------------------------
# Device kernel
# ---------------------------------------------------------------------------

def build(stage="full"):
    nc = bacc.Bacc("TRN2", target_bir_lowering=False, debug=False,
                   num_devices=N_CORES)
    prm = {n: nc.declare_dram_parameter(n, sh, dt, isOutput=False)
           for n, sh, dt in INPUT_SPECS}
    if stage == "A":
        out_prm = nc.declare_dram_parameter("out_a", [514, VCOLS], f32,
                                            isOutput=True)
    elif stage.startswith("B"):
        out_prm = nc.declare_dram_parameter("out_b", [128, 2048], f16,
                                            isOutput=True)
    else:
        out_prm = nc.declare_dram_parameter("out", [128, 2, 16, 32], f32,
                                            isOutput=True)
    if stage == "dbg":
        for n, sh, dt in [("dbg_xraw", [128, 4, XR, 34], f16),
                          ("dbg_xb", [128, 2, 16, 34], f16),
                          ("dbg_xbfull", [128, 2, 32, 32], f16),
                          ("dbg_gate", [128, 2, 1], f32),
                          ("dbg_sig", [1, 1024], f16),
                          ("dbg_fused", [128, 2, 32, 32], f16),
                          ("dbg_pd", [64, 85], f32),
                          ("dbg_pri0", [128, 512], f16),
                          ("dbg_pri1", [128, 512], f16)]:
            prm[n] = nc.declare_dram_parameter(n, sh, dt, isOutput=True)
    with tile.TileContext(nc) as tc:
        _emit(tc, nc, prm, stage, out_prm)
    nc.compile()
    return nc


def _emit(tc, nc, prm, stage, out_prm):
    import contextlib
    es = contextlib.ExitStack()
    with es:
        wpool = es.enter_context(tc.tile_pool(name="wpool", bufs=1))
        apool = es.enter_context(tc.tile_pool(name="apool", bufs=1))
        dram = es.enter_context(tc.tile_pool(name="dram", bufs=1, space="DRAM"))
        aonly_cm = tc.tile_pool(name="aonly", bufs=1)
        aonly = aonly_cm.__enter__()

        def load(name, pool=wpool):
            t = pool.tile(list(prm[name].shape), prm[name].dtype,
                          name=f"{name}_sb")
            nc.sync.dma_start(t[:], prm[name][:])
            return t

        h_reg = nc.vector.partition_id() % 2
        my_off = h_reg * 448          # my val-window start col in [0, 1024)
        xoff = 2 - h_reg * 2          # x-window row where val data lands
        h_act = nc.scalar.partition_id() % 2        # Act-side copy for DMAs
        my_off_a = h_act * 448
        send_off = 448 - h_act * 448  # peer's val-window start col
        r0v = (nc.tensor.partition_id() % 2) * 16   # PE-side own-row base

        pk_wT = load("pk_wT")
        pkb2 = load("pkb2")
        xm_sb = aonly.tile([128, 2, 4, 34, 34], f16, name="xm_sb")
        for t in range(4):
            nc.sync.dma_start(xm_sb[:, :, t, 0:18, :],
                              prm["xm"][:, :, t, 0:18, :])
            nc.sync.dma_start(xm_sb[:, :, t, 18:34, :],
                              prm["xm"][:, :, t, 18:34, :])
        xq_sb = load("xq", aonly)
        vT_sb = load("vT", aonly)

        # pre-zero the comp frame in DRAM (reused much later by the 7x7 conv)
        comp_d = dram.tile([2, 38, 38], f16)
        zz = wpool.tile([2, 38 * 38], f16, name="zz")
        nc.vector.memset(zz[:], 0.0)
        nc.sync.dma_start(comp_d.rearrange("s r c -> s (r c)"), zz[:, :])

        ones_1pA = wpool.tile([1, 128], f16, name="ones_1pA")
        nc.vector.memset(ones_1pA[:, :], 1.0)
        warm_rowA = wpool.tile([1, 512], f16, name="warm_rowA")
        nc.vector.memset(warm_rowA[:, :], 1.0)

        # ================= phase A =================
        mk_sb = aonly.tile([128, 2, 1024], f16)
        qk_sb = aonly.tile([128, 1024], f16)

        with tc.tile_pool(name="psA", bufs=2, space="PSUM") as psA:
            pswA = psA.tile([128, 512], f32, name="pswA")
            for _ in range(8):
                nc.tensor.matmul(pswA[:, :], ones_1pA[:, :], warm_rowA[:, :],
                                 start=True, stop=True)
            wsbA = wpool.tile([1, 4], f32, name="wsbA")
            nc.vector.tensor_copy(wsbA[:, :], pswA[0:1, 0:4])
            wscA = dram.tile([1, 4], f32)
            nc.sync.dma_start(wscA[:], wsbA[:, :])
            for tp in range(2):
                for n in range(2):
                    pm = psA.tile([128, 512], f32, tag="mkps", name="pm")
                    for par in range(2):
                        t = 2 * tp + par
                        k = 0
                        for j in range(2):
                            for dy in range(3):
                                for dx in range(3):
                                    nc.tensor.matmul(
                                        pm[64 * par: 64 * par + 64, :],
                                        pk_wT[:, j, dy, dx, :],
                                        xm_sb[:, j, t,
                                              n * 16 + dy: n * 16 + dy + 16,
                                              dx: dx + 32],
                                        start=(k == 0), stop=(k == 17),
                                        tile_position=(0, 64 * par),
                                    )
                                    k += 1
                    nc.scalar.activation(
                        mk_sb[:, tp, n * 512: (n + 1) * 512], pm[:, :],
                        AF.Identity, bias=pkb2[:, 0:1])

            for n in range(2):
                pq = psA.tile([64, 512], f32, tag="qkps", name="pq")
                k = 0
                for j in range(2):
                    for dy in range(3):
                        for dx in range(3):
                            nc.tensor.matmul(
                                pq[:, :], pk_wT[:, j, dy, dx, :],
                                xq_sb[:, j, n * 16 + dy: n * 16 + dy + 16,
                                      dx: dx + 32],
                                start=(k == 0), stop=(k == 17))
                            k += 1
                nc.scalar.activation(
                    qk_sb[0:64, n * 512: (n + 1) * 512], pq[:, :],
                    AF.Identity, bias=pkb2[0:64, 0:1])
            # replicate qk to partitions 64..127 so odd-frame mk slices
            # (base partition 64) can stream against it
            nc.sync.dma_start(qk_sb[64:128, :], qk_sb[0:64, :])

        arv = dram.tile([257, VCOLS], bf)
        arvg = dram.tile([2, 257, VCOLS], bf)

        v_sb = wpool.tile([128, 2, 1024], bf, name="v_sb")
        s_sb = wpool.tile([1, 1024], bf, name="s_sb")
        sd = dram.tile([1, VCOLS], bf)

        with (
            tc.tile_pool(name="psAff", bufs=2, space="PSUM") as psAff,
            tc.tile_pool(name="psV", bufs=1, space="PSUM") as psV,
        ):
            vps = [psV.tile([128, 1024], f32, name=f"vps{j}") for j in range(2)]
            s_acc = aonly.tile([128, 1024], f32, name="s_acc")

            order = [16 * hh + o + 8 * par for hh in range(2) for o in range(8)
                     for par in range(2)]

            def emit_value(idx, i, e_t):
                for j in range(2):
                    for qn in range(2):
                        nc.tensor.matmul(
                            vps[j][:, qn * 512: (qn + 1) * 512],
                            vT_sb[:, i, j * 128: (j + 1) * 128],
                            e_t[:, qn * 512: (qn + 1) * 512],
                            start=(idx == 0), stop=(idx == 31))
                if idx == 0:
                    nc.vector.tensor_copy(s_acc[:, :], e_t[:, :])
                else:
                    nc.vector.tensor_add(s_acc[:, :], s_acc[:, :], e_t[:, :])

            # software-pipelined: frame i's value matmuls run one iteration
            # late so they never wait on the exp() of the same frame
            prev = None
            for idx, i in enumerate(order):
                t = i >> 3
                pb = i & 7
                tp, par = t >> 1, t & 1
                lhs_aff = mk_sb[64 * par: 64 * par + 64, tp,
                                pb * 128: pb * 128 + 128]
                e_t = aonly.tile([128, 1024], bf, tag="e", name="e_t", bufs=3)
                for qn in range(2):
                    pa = psAff.tile([128, 512], f32, tag="affp", name="pa")
                    nc.tensor.matmul(
                        pa[:, :], lhs_aff,
                        qk_sb[64 * par: 64 * par + 64,
                              qn * 512: (qn + 1) * 512],
                        start=True, stop=True)
                    nc.scalar.activation(
                        e_t[:, qn * 512: (qn + 1) * 512], pa[:, :],
                        AF.Exp, scale=0.125)
                if prev is not None:
                    emit_value(*prev)
                prev = (idx, i, e_t)
            emit_value(*prev)

            for j in range(2):
                nc.vector.tensor_copy(v_sb[:, j, :], vps[j][:, :])
                nc.scalar.dma_start(arv[128 * j: 128 * j + 128, :],
                                    v_sb[:, j, bass.ds(send_off, VCOLS)])
            # fold the 128-partition sumexp accumulator with a ones matmul
            ones_f32 = aonly.tile([128, 1], f32, name="ones_f32")
            nc.vector.memset(ones_f32[:], 1.0)
            sfold = psV.tile([1, 1024], f32, tag="sfold", name="sfold")
            for qn in range(2):
                nc.tensor.matmul(sfold[0:1, qn * 512: (qn + 1) * 512],
                                 ones_f32[:, 0:1],
                                 s_acc[:, qn * 512: (qn + 1) * 512],
                                 start=True, stop=True)
            nc.vector.tensor_copy(s_sb[:, :], sfold[:, :])
            nc.scalar.dma_start(arv[256:257, :],
                                s_sb[0:1, bass.ds(send_off, VCOLS)])
            nc.scalar.dma_start(sd[:], s_sb[0:1, bass.ds(my_off_a, VCOLS)])

        nc.gpsimd.collective_compute(
            "AllGather", ALU.bypass, replica_groups=PAIRS,
            ins=[arv[:].opt()], outs=[arvg[:].opt()])

        aonly_cm.__exit__(None, None, None)

        if stage == "A":
            with tc.tile_pool(name="cmb", bufs=1) as cmb:
                for sl in range(2):
                    ca = cmb.tile([128, VCOLS], bf, tag="ca", name="ca")
                    co = cmb.tile([128, VCOLS], f32, tag="co", name="co")
                    for part in range(2):
                        nc.sync.dma_start(
                            ca[:, :], arvg[sl, 128 * part: 128 * part + 128, :])
                        nc.vector.tensor_copy(co[:, :], ca[:, :])
                        nc.sync.dma_start(
                            out_prm[257 * sl + 128 * part:
                                    257 * sl + 128 * part + 128, :], co[:, :])
                    cs = cmb.tile([1, VCOLS], bf, tag="cs", name="cs")
                    cso = cmb.tile([1, VCOLS], f32, tag="cso", name="cso")
                    nc.sync.dma_start(cs[:], arvg[sl, 256:257, :])
                    nc.vector.tensor_copy(cso[:, :], cs[:, :])
                    nc.sync.dma_start(out_prm[257 * sl + 256:
                                              257 * sl + 257, :], cso[:, :])
            return

        # ================= phase B =================
        wk = es.enter_context(tc.tile_pool(name="wk", bufs=1))
        rb1_wT = load("rb1_wT")
        rb2_wT = load("rb2_wT")
        rbd_wT = load("rbd_wT")
        rb1_b = load("rb1_b")
        xb_bias = load("xb_bias")
        mlp_w1T = load("mlp_w1T")
        mlp_b1 = load("mlp_b1")
        mlp_w2T = load("mlp_w2T")
        mlp_b2x2 = load("mlp_b2x2")
        spw_r = load("spw_r")
        bn_sb = load("bn_sb")
        psp_wT = load("psp_wT")
        Wup = load("Wup")
        bott_wT = load("bott_wT")
        bott_b = load("bott_b")
        ident = load("ident")
        ident16 = load("ident16")
        slotmask = load("slotmask")

        # preload the Sigmoid act table off the critical path
        warm_sig = wk.tile([1, 2], f32, name="warm_sig")
        nc.scalar.activation(warm_sig[0:1, 0:1], bn_sb[0:1, 0:1], AF.Sigmoid)

        # ---- x window tiles; q-halves from host, independent of the AG ----
        x_raw = apool.tile([128, 4, XR, 34], f16)
        x_relu = apool.tile([128, 4, XR, 34], f16)
        for tt in (x_raw, x_relu):
            nc.vector.memset(tt[:, 2:4, :, :], 0.0)
        nc.sync.dma_start(x_raw[:, 0:2, :, :], prm["xqb_raw"][:])
        nc.sync.dma_start(x_relu[:, 0:2, :, :], prm["xqb_relu"][:])

        ones_1p = wk.tile([1, 128], f16, name="ones_1p")
        nc.vector.memset(ones_1p[:, :], 1.0)
        warm_row = wk.tile([1, 512], f16, name="warm_row")
        nc.vector.memset(warm_row[:, :], 1.0)

        r1_relu = apool.tile([128, 2, RR, 34], f16)
        nc.vector.memset(r1_relu[:, :, :, 0:1], 0.0)
        nc.vector.memset(r1_relu[:, :, :, 33:34], 0.0)
        rmaskb = apool.tile([128, RR, 34], f16)
        nc.sync.dma_start(rmaskb[:], prm["rmask"][:].partition_broadcast(128))
        xbv = apool.tile([128, 2, 16, 34], f16)
        nc.vector.memset(xbv[:, :, :, 0:1], 0.0)
        nc.vector.memset(xbv[:, :, :, 33:34], 0.0)

        with tc.tile_pool(name="psB", bufs=1, space="PSUM") as psB:
            prs = {(m, gi): psB.tile([128, 288], f32, name=f"pr{m}{gi}")
                   for m in range(2) for gi in range(2)}
            pxs = {m: psB.tile([128, 512], f32, name=f"px{m}")
                   for m in range(2)}

            # --- q-chunk partial sums: run while the value AllGather flies ---
            for m in range(2):
                for gi in range(2):
                    pr = prs[(m, gi)]
                    g0 = 9 * gi
                    k = 0
                    for j in range(2):
                        for dy in range(3):
                            for dx in range(3):
                                nc.tensor.matmul(
                                    pr[:, :],
                                    rb1_wT[:, j, dy, dx,
                                           m * 128: m * 128 + 128],
                                    x_relu[:, j, g0 + dy: g0 + dy + 9,
                                           dx: dx + 32],
                                    start=(k == 0), stop=False)
                                k += 1
            for m in range(2):
                px = pxs[m]
                k = 0
                for j in range(2):
                    for dy in range(3):
                        for dx in range(3):
                            nc.tensor.matmul(
                                px[:, :],
                                rbd_wT[:, j, dy, dx, m * 128: m * 128 + 128],
                                x_raw[:, j, 1 + dy: 1 + dy + 16, dx: dx + 32],
                                start=(k == 0), stop=False)
                            k += 1

            # p-state warmers bridging the val-AllGather wait, so the
            # val-chunk convs dispatch against a busy PE (full clock). The
            # scratch psum is DMA'd to a dram scrap so the writes are live.
            psw = psB.tile([128, 512], f32, name="psw")
            for _ in range(115):
                nc.tensor.matmul(psw[:, :], ones_1p[:, :], warm_row[:, :],
                                 start=True, stop=True)
            wsb = wk.tile([1, 4], f32, name="wsb")
            nc.vector.tensor_copy(wsb[:, :], psw[0:1, 0:4])
            wsc = dram.tile([1, 4], f32)
            nc.sync.dma_start(wsc[:], wsb[:, :])

            # ---- val: combine AG slots (masked select), normalize ----
            xoff_a = 2 - h_act * 2
            h_g = nc.gpsimd.partition_id() % 2
            xoff_g = 2 - h_g * 2
            vs0 = wk.tile([128, 2, VCOLS], bf, name="vs0")
            vs1 = wk.tile([128, 2, VCOLS], bf, name="vs1")
            s0b = wk.tile([128, VCOLS], bf, name="s0b")
            s1b = wk.tile([128, VCOLS], bf, name="s1b")
            slb = wk.tile([128, VCOLS], bf, name="slb")
            nc.scalar.dma_start(s0b[:],
                                arvg[0, 256:257, :].partition_broadcast(128))
            nc.gpsimd.dma_start(s1b[:],
                                arvg[1, 256:257, :].partition_broadcast(128))
            nc.sync.dma_start(slb[:], sd[:].partition_broadcast(128))
            nc.sync.dma_start(vs0[:, 0, :], arvg[0, 0:128, :])
            nc.scalar.dma_start(vs0[:, 1, :], arvg[0, 128:256, :])
            nc.gpsimd.dma_start(vs1[:, 0, :], arvg[1, 0:128, :])
            nc.sync.dma_start(vs1[:, 1, :], arvg[1, 128:256, :])
            s_tot = wk.tile([128, VCOLS], f32, name="s_tot")
            nc.vector.scalar_tensor_tensor(
                s_tot[:, :], s0b[:, :], slotmask[:, 0:1], slb[:, :],
                ALU.mult, ALU.add)
            nc.vector.scalar_tensor_tensor(
                s_tot[:, :], s1b[:, :], slotmask[:, 1:2], s_tot[:, :],
                ALU.mult, ALU.add)
            inv = wk.tile([128, VCOLS], f32, name="inv")
            nc.vector.reciprocal(inv[:, :], s_tot[:, :])

            v_win = wk.tile([128, 2, VCOLS], f32, name="v_win")
            nc.vector.scalar_tensor_tensor(
                v_win[:, :, :], vs0[:, :, :], slotmask[:, 0:1],
                v_sb[:, :, bass.ds(my_off, VCOLS)], ALU.mult, ALU.add)
            nc.vector.scalar_tensor_tensor(
                v_win[:, :, :], vs1[:, :, :], slotmask[:, 1:2],
                v_win[:, :, :], ALU.mult, ALU.add)
            for j in range(2):
                nc.vector.tensor_mul(
                    x_raw[:, 2 + j, bass.ds(xoff, 18), 1:33],
                    v_win[:, j, :].rearrange("p (r c) -> p r c", c=32),
                    inv[:, :].rearrange("p (r c) -> p r c", c=32))
                nc.scalar.activation(
                    x_relu[:, 2 + j, bass.ds(xoff_a, 18), 1:33],
                    x_raw[:, 2 + j, bass.ds(xoff_a, 18), 1:33], AF.Relu)

            # --- val-chunk completion of rb1, then rbd + rb2 ---
            for m in range(2):
                for gi in range(2):
                    pr = prs[(m, gi)]
                    g0 = 9 * gi
                    k = 0
                    for j in range(2, 4):
                        for dy in range(3):
                            for dx in range(3):
                                nc.tensor.matmul(
                                    pr[:, :],
                                    rb1_wT[:, j, dy, dx,
                                           m * 128: m * 128 + 128],
                                    x_relu[:, j, g0 + dy: g0 + dy + 9,
                                           dx: dx + 32],
                                    start=False, stop=(k == 17))
                                k += 1
                    nc.scalar.activation(
                        r1_relu[:, m, g0: g0 + 9, 1:33], pr[:, :],
                        AF.Relu, bias=rb1_b[:, m: m + 1])
                    nc.vector.tensor_mul(r1_relu[:, m, g0: g0 + 9, 1:33],
                                         r1_relu[:, m, g0: g0 + 9, 1:33],
                                         rmaskb[:, g0: g0 + 9, 1:33])

            for m in range(2):
                px = pxs[m]
                k = 0
                for j in range(2, 4):
                    for dy in range(3):
                        for dx in range(3):
                            nc.tensor.matmul(
                                px[:, :],
                                rbd_wT[:, j, dy, dx, m * 128: m * 128 + 128],
                                x_raw[:, j, 1 + dy: 1 + dy + 16, dx: dx + 32],
                                start=False, stop=False)
                            k += 1
                for j in range(2):
                    for dy in range(3):
                        for dx in range(3):
                            nc.tensor.matmul(
                                px[:, :],
                                rb2_wT[:, j, dy, dx, m * 128: m * 128 + 128],
                                r1_relu[:, j, dy: dy + 16, dx: dx + 32],
                                start=False, stop=(k == 35))
                            k += 1
                nc.scalar.activation(
                    xbv[:, m, :, 1:33], px[:, :],
                    AF.Identity, bias=xb_bias[:, m: m + 1])

        if stage == "dbg":
            nc.sync.dma_start(prm["dbg_xraw"][:], x_raw[:])
            nc.sync.dma_start(prm["dbg_xb"][:], xbv[:])
        if stage == "B1":
            nc.sync.dma_start(out_prm[:, 0:1088],
                              xbv.rearrange("p j r c -> p (j r c)"))
            return

        # ---- own-row xb exchange: slot s lands at image rows 16s..16s+15 ----
        xbd = dram.tile([128, 2, 16, 32], f16)
        xbg = dram.tile([2, 128, 2, 16, 32], f16)
        nc.sync.dma_start(xbd[:, 0], xbv[:, 0, :, 1:33])
        nc.scalar.dma_start(xbd[:, 1], xbv[:, 1, :, 1:33])
        nc.gpsimd.collective_compute(
            "AllGather", ALU.bypass, replica_groups=PAIRS,
            ins=[xbd[:].opt()], outs=[xbg[:].opt()])
        with tc.tile_pool(name="psW2", bufs=1, space="PSUM") as psW2:
            psw2 = psW2.tile([128, 512], f32, name="psw2")
            for _ in range(190):
                nc.tensor.matmul(psw2[:, :], ones_1p[:, :], warm_row[:, :],
                                 start=True, stop=True)
            wsb2 = wk.tile([1, 4], f32, name="wsb2")
            nc.vector.tensor_copy(wsb2[:, :], psw2[0:1, 0:4])
            wsc2 = dram.tile([1, 4], f32)
            nc.sync.dma_start(wsc2[:], wsb2[:, :])
        xb_full = apool.tile([128, 2, 32, 32], f16)
        nc.sync.dma_start(xb_full[:, :, 0:16, :], xbg[0])
        nc.scalar.dma_start(xb_full[:, :, 16:32, :], xbg[1])
        xf = xb_full.rearrange("p j r c -> p j (r c)")

        if stage == "dbg":
            nc.sync.dma_start(prm["dbg_xbfull"][:], xb_full[:])
        if stage == "B2":
            nc.sync.dma_start(out_prm[:, 0:2048],
                              xb_full.rearrange("p j r c -> p (j r c)"))
            return

        # ---- CBAM channel gate (local now: stats over the full image) ----
        gate_in = wk.tile([128, 2, 2], f32, name="gate_in")
        stat_s = wk.tile([128, 2, 1], f32, name="stat_s")
        max16 = wk.tile([128, 2, 1], f16, name="max16")
        for j in range(2):
            nc.vector.tensor_reduce(max16[:, j, :], xf[:, j, :], AX.X,
                                    ALU.max)
            nc.vector.tensor_reduce(stat_s[:, j, :], xf[:, j, :], AX.X, ALU.add)
        nc.scalar.copy(gate_in[:, :, 1:2], max16[:, :, :])
        nc.scalar.mul(gate_in[:, :, 0:1], stat_s[:, :, :], 1.0 / 1024.0)

        # transposed copies of xb for the per-pixel channel max
        # chunk idx = slot*8 + j*4 + k  (k = 4-row group within the slot)
        xbT = wk.tile([128, 16, 128], f16, name="xbT")
        with tc.tile_pool(name="psT", bufs=4, space="PSUM") as psT:
            for idx in range(16):
                sl, j, kk = idx >> 3, (idx >> 2) & 1, idx & 3
                pt = psT.tile([128, 128], f16, tag="pt", name="pt")
                nc.tensor.transpose(
                    pt[:, :],
                    xf[:, j, 512 * sl + 128 * kk: 512 * sl + 128 * kk + 128],
                    ident16[:, :])
                if idx % 2 == 0:
                    nc.vector.tensor_copy(xbT[:, idx, :], pt[:, :])
                else:
                    nc.scalar.copy(xbT[:, idx, :], pt[:, :])

        gate = wk.tile([128, 2, 1], f32, name="gate")
        with tc.tile_pool(name="psG", bufs=1, space="PSUM") as psG:
            ph1 = psG.tile([16, 2], f32, name="ph1")
            for j in range(2):
                nc.tensor.matmul(ph1[:, :], mlp_w1T[:, j, :], gate_in[:, j, :],
                                 start=(j == 0), stop=(j == 1))
            h1 = wk.tile([16, 2], f32, name="h1")
            nc.scalar.activation(h1[:, :], ph1[:, :], AF.Relu,
                                 bias=mlp_b1[:, 0:1])
            for j in range(2):
                ph2 = psG.tile([128, 2], f32, tag="ph2", name="ph2")
                nc.tensor.matmul(ph2[:, :], mlp_w2T[:, j * 128: j * 128 + 128],
                                 h1[:, :], start=True, stop=True)
                h2 = wk.tile([128, 2], f32, tag="h2", name="h2")
                nc.vector.tensor_copy(h2[:, :], ph2[:, :])
                t2 = wk.tile([128, 1], f32, tag="t2", name="t2")
                nc.vector.tensor_add(t2[:, :], h2[:, 0:1], h2[:, 1:2])
                nc.scalar.activation(gate[:, j, :], t2[:, :], AF.Sigmoid,
                                     bias=mlp_b2x2[:, j: j + 1])

        if stage == "dbg":
            nc.sync.dma_start(prm["dbg_gate"][:], gate[:])
        if stage == "B3":
            g16 = wk.tile([128, 2, 1], f16, name="g16b3")
            nc.scalar.copy(g16[:, :, :], gate[:, :, :])
            nc.sync.dma_start(out_prm[:, 0:2], g16[:, :, 0])
            return

        # gate broadcast along pixels via PE: transpose the [128, 2] gate to
        # [2, 128], then ones-matmuls replicate each j-row across partitions
        gate16 = wk.tile([128, 2], f16, name="gate16")
        nc.scalar.copy(gate16[:, :], gate[:, :, 0])
        gT = wk.tile([2, 128], f16, name="gT")
        sel = load("sel")
        gb4 = [wk.tile([128, 4, 128], f16, tag=f"gb4{j}", name=f"gb4{j}")
               for j in range(2)]
        with tc.tile_pool(name="psGB", bufs=2, space="PSUM") as psGB:
            ptg = psGB.tile([2, 128], f16, tag="ptg", name="ptg")
            nc.tensor.transpose(ptg[:, :], gate16[:, :], ident16[:, :])
            nc.vector.tensor_copy(gT[:, :], ptg[:, :])
            for j in range(2):
                pgb = psGB.tile([128, 128], f32, tag="pgb", name="pgb")
                nc.tensor.matmul(pgb[:, :], sel[:, 128 * j: 128 * j + 128],
                                 gT[:, :], start=True, stop=True)
                for kk in range(4):
                    eng = nc.vector if kk % 2 == 0 else nc.scalar
                    if kk % 2 == 0:
                        eng.tensor_copy(gb4[j][:, kk, :], pgb[:, :])
                    else:
                        eng.copy(gb4[j][:, kk, :], pgb[:, :])
        gate_sc = wk.tile([128, 2, 1], f16, name="gate_sc")
        nc.scalar.mul(gate_sc[:, :, :], gate[:, :, :], 1.0 / 256.0)
        if stage == "B3a":
            nc.sync.dma_start(out_prm[:, 0:512],
                              gb4[0].rearrange("p k c -> p (k c)"))
            return

        # per-pixel channel max of xb*gate from the transposed copies
        scrs = [wk.tile([128, 4, 128], f16, tag=f"scr{i}", name=f"scr{i}")
                for i in range(2)]
        cm16 = wk.tile([128, 16, 1], f16, name="cm16")
        for sl in range(2):
            for j in range(2):
                base = sl * 8 + j * 4
                scr = scrs[(sl * 2 + j) % 2]
                nc.vector.tensor_mul(scr[:, :, :], xbT[:, base: base + 4, :],
                                     gb4[j][:, :, :])
                nc.vector.tensor_reduce(cm16[:, base: base + 4, :],
                                        scr[:, :, :], AX.X, ALU.max)
        cmax = wk.tile([128, 8, 1], f16, name="cmax")
        c4 = cm16.rearrange("p (s j k) o -> p s j k o", s=2, j=2)
        nc.vector.tensor_max(cmax.rearrange("p (s k) o -> p s k o", s=2),
                             c4[:, :, 0], c4[:, :, 1])
        if stage == "B3b":
            nc.sync.dma_start(out_prm[:, 0:8], cmax[:, :, 0])
            return

        # per-pixel channel mean of xb*gate via gate-weighted ones-matmul
        mean_sb = wk.tile([1, 1024], f16, name="mean_sb")
        with tc.tile_pool(name="psM", bufs=1, space="PSUM") as psM:
            for half in range(2):
                pm1 = psM.tile([1, 512], f32, tag="pm1", name="pm1")
                for j in range(2):
                    nc.tensor.matmul(pm1[0:1, :],
                                     gate_sc[:, j, :],
                                     xf[:, j, half * 512: half * 512 + 512],
                                     start=(j == 0), stop=(j == 1))
                nc.scalar.copy(mean_sb[:, half * 512: half * 512 + 512],
                               pm1[:, :])

        if stage == "B3c":
            nc.sync.dma_start(out_prm[0:1, 0:1024], mean_sb[:, :])
            return

        # assemble comp = [max, mean] into the padded DRAM frame, 7x7 conv.
        # cmax partition p = (a*32+b) maps to comp rows 4k+a, col b.
        engs = [nc.sync, nc.scalar, nc.gpsimd]
        for k in range(8):
            engs[k % 3].dma_start(
                bass.AP(comp_d.tensor, 117 + 152 * k, [[38, 4], [1, 32]]),
                cmax[:, k: k + 1, 0])
        nc.scalar.dma_start(
            bass.AP(comp_d.tensor, 1444 + 117, [[38, 32], [1, 32]]),
            mean_sb[:, :])
        il = wk.tile([14, 32, 38], f16, name="il")
        nc.sync.dma_start(
            il[:, :, :],
            bass.AP(comp_d.tensor, 0,
                    [[1444, 2], [38, 7], [38, 32], [1, 38]]))
        # p-state warmers: keep PE busy through the comp DMA latency so the
        # 7x7 conv runs at full clock. They accumulate into the psum tile the
        # sigb broadcast later overwrites (start=True), so they aren't dead.
        psSB_cm = tc.tile_pool(name="psSB", bufs=2, space="PSUM")
        psSB = psSB_cm.__enter__()
        psb_t = [psSB.tile([128, 512], f32, tag="psb", name=f"psb{i}")
                 for i in range(2)]
        for _ in range(40):
            nc.tensor.matmul(psb_t[0][:, :], ones_1p[:, :], warm_row[:, :],
                             start=True, stop=True)
        if stage == "B3d":
            nc.sync.dma_start(out_prm[0:14, 0:1216],
                              il.rearrange("p r c -> p (r c)"))
            return
        sig = wk.tile([1, 1024], f16, name="sig")
        with tc.tile_pool(name="psS", bufs=2, space="PSUM") as psS:
            for half in range(2):
                pss = psS.tile([1, 512], f32, tag="pss", name="pss")
                for dx in range(7):
                    nc.tensor.matmul(pss[:, :], spw_r[:, dx: dx + 1],
                                     il[:, 16 * half: 16 * half + 16,
                                        dx: dx + 32],
                                     start=(dx == 0), stop=(dx == 6))
                nc.scalar.activation(sig[:, half * 512: half * 512 + 512],
                                     pss[:, :], AF.Sigmoid,
                                     scale=bn_sb[0:1, 0:1], bias=bn_sb[0:1, 1:2])
        # sig broadcast across partitions via ones-matmul (no DRAM round trip)
        sigb = wk.tile([128, 1024], f16, name="sigb")
        for half in range(2):
            psb = psb_t[half]
            nc.tensor.matmul(psb[:, :], ones_1p[:, :],
                             sig[:, half * 512: half * 512 + 512],
                             start=True, stop=True)
            if half == 0:
                nc.vector.tensor_copy(sigb[:, 0:512], psb[:, :])
            else:
                nc.scalar.copy(sigb[:, 512:1024], psb[:, :])
        psSB_cm.__exit__(None, None, None)

        if stage == "dbg":
            nc.sync.dma_start(prm["dbg_sig"][:], sig[:])
        if stage == "B4":
            nc.sync.dma_start(out_prm[0:1, 0:1024], sig[:, :])
            return

        # fused = xb * (1 + gate * sig)   (full image, f16)
        fused = apool.tile([128, 2, 32, 32], f16)
        fv = fused.rearrange("p j r c -> p j (r c)")
        xcb = wk.tile([128, 2, 1024], f16, name="xcb")
        tmb = wk.tile([128, 2, 1024], f16, name="tmb")
        for j in range(2):
            nc.scalar.mul(xcb[:, j, :], xf[:, j, :], gate[:, j, 0:1])
            nc.vector.tensor_mul(tmb[:, j, :], xcb[:, j, :], sigb[:, :])
            nc.vector.tensor_add(fv[:, j, :], xf[:, j, :], tmb[:, j, :])

        if stage == "dbg":
            nc.sync.dma_start(prm["dbg_fused"][:], fused[:])

        # ---- PSP pools (full-image pyramid, local) ----
        pools = wk.tile([128, 2, 85], f16, name="pools")
        with nc.allow_low_precision("pool pyramid partials; ~0.1% rel err"):
            for j in range(2):
                f8 = fused[:, j].rearrange(
                    "p (rb ri) (cb ci) -> p rb cb ri ci", ri=4, ci=4)
                p8v = pools[:, j, 21:85].rearrange("p (rb cb) -> p rb cb",
                                                   cb=8)
                nc.vector.tensor_reduce(p8v, f8, AX.XY, ALU.add)
                p8i = pools[:, j, 21:85].rearrange(
                    "p (rb ri cb ci) -> p rb cb ri ci", rb=4, ri=2, cb=4, ci=2)
                p4v = pools[:, j, 5:21].rearrange("p (rb cb) -> p rb cb",
                                                  cb=4)
                nc.vector.tensor_reduce(p4v, p8i, AX.XY, ALU.add)
                p4i = pools[:, j, 5:21].rearrange(
                    "p (rb ri cb ci) -> p rb cb ri ci", rb=2, ri=2, cb=2, ci=2)
                p2v = pools[:, j, 1:5].rearrange("p (rb cb) -> p rb cb", cb=2)
                nc.vector.tensor_reduce(p2v, p4i, AX.XY, ALU.add)
            nc.vector.tensor_reduce(pools[:, :, 0:1], pools[:, :, 1:5], AX.X,
                                    ALU.add)

        psO_cm = tc.tile_pool(name="psO", bufs=2, space="PSUM")
        psO = psO_cm.__enter__()
        po_warm = psO.tile([128, 512], f32, tag="po", name="po")

        # 1x1 convs on pools: pd [64, 85], column layout [s8|s1|s4|s2]
        pd = wk.tile([64, 85], f32, name="pd")
    # (pools pyramid accumulates in f16; ~0.1% rel err, inside budget)
        with tc.tile_pool(name="psP", bufs=1, space="PSUM") as psP:
            pd_ps = psP.tile([64, 85], f32, name="pd_ps")
            for s in (8, 1, 4, 2):
                po, qo, n = PDOFF[s], POFF[s], s * s
                for j in range(2):
                    nc.tensor.matmul(pd_ps[:, po: po + n],
                                     psp_wT[:, j, SI[s], :],
                                     pools[:, j, qo: qo + n],
                                     start=(j == 0), stop=(j == 1))
            nc.scalar.copy(pd[:, :], pd_ps[:, :])

        if stage == "dbg":
            nc.sync.dma_start(prm["dbg_pd"][:], pd[:])

        # upsample via PE transpose of pd + folded operators Wup (bf16)
        ones_f = wk.tile([128, 512], bf, name="ones_f")
        nc.vector.memset(ones_f[:], 1.0)
        pdT_a = wk.tile([65, 64], bf, name="pdT_a")
        pdT_b = wk.tile([16, 64], bf, name="pdT_b")
        pdT_c = wk.tile([4, 64], bf, name="pdT_c")
        pri = [wk.tile([128, 512], f16, tag=f"pri{i}", name=f"pri{i}")
               for i in range(2)]
        with tc.tile_pool(name="psQ", bufs=2, space="PSUM") as psQ:
            pta = psQ.tile([65, 64], f32, tag="pta", name="pta")
            nc.tensor.transpose(pta[:, :], pd[:, 0:65], ident[0:64, 0:64])
            nc.scalar.activation(pdT_a[:, :], pta[:, :], AF.Identity,
                                 scale=0.0625)
            ptb = psQ.tile([16, 64], f32, tag="ptb", name="ptb")
            nc.tensor.transpose(ptb[:, :], pd[:, 65:81], ident[0:64, 0:64])
            nc.scalar.activation(pdT_b[:, :], ptb[:, :], AF.Identity,
                                 scale=0.0625)
            ptc = psQ.tile([4, 64], f32, tag="ptc", name="ptc")
            nc.tensor.transpose(ptc[:, :], pd[:, 81:85], ident[0:64, 0:64])
            nc.scalar.activation(pdT_c[:, :], ptc[:, :], AF.Identity,
                                 scale=0.0625)
        with tc.tile_pool(name="psR", bufs=2, space="PSUM") as psR:
            pp0 = psR.tile([128, 512], f32, tag="pp", name="pp0")
            nc.tensor.matmul(pp0[0:64, :], pdT_a[64:65, :], ones_f[64:65, :],
                             start=True, stop=True, tile_position=(64, 0))
            nc.tensor.matmul(pp0[64:128, :], pdT_c[0:4, :], Wup[0:4, 0, :],
                             start=True, stop=True, tile_position=(0, 64))
            nc.scalar.copy(pri[0][:, :], pp0[:, :])
            pp1 = psR.tile([128, 512], f32, tag="pp", name="pp1")
            nc.tensor.matmul(pp1[0:64, :], pdT_b[0:16, :], Wup[0:16, 1, :],
                             start=True, stop=True)
            nc.tensor.matmul(pp1[64:128, :], pdT_a[0:64, :], Wup[0:64, 2, :],
                             start=True, stop=True, tile_position=(0, 64))
            nc.scalar.copy(pri[1][:, :], pp1[:, :])

        if stage == "dbg":
            nc.sync.dma_start(prm["dbg_pri0"][:], pri[0][:])
            nc.sync.dma_start(prm["dbg_pri1"][:], pri[1][:])
        if stage == "B5":
            nc.sync.dma_start(out_prm[:, 0:512], pri[0][:])
            return

        out_sb = wk.tile([128, 2, 512], f32, name="out_sb")
        fown = fused[:, :, bass.ds(r0v, 16), :]
        rhs_chunks = [(2, fown[:, 0].rearrange("p r c -> p (r c)")),
                      (3, fown[:, 1].rearrange("p r c -> p (r c)")),
                      (0, pri[0][:, :]), (1, pri[1][:, :])]
        for m in range(2):
            po = po_warm if m == 0 else psO.tile([128, 512], f32, tag="po",
                                                 name="po")
            for i, (k, rhs) in enumerate(rhs_chunks):
                nc.tensor.matmul(po[:, :],
                                 bott_wT[:, k, m * 128: m * 128 + 128],
                                 rhs,
                                 start=(i == 0), stop=(i == 3))
            nc.scalar.activation(out_sb[:, m, :], po[:, :], AF.Relu,
                                 bias=bott_b[:, m: m + 1])
            nc.sync.dma_start(
                out_prm[:, m],
                out_sb[:, m, :].rearrange("p (r c) -> p r c", c=32))
        psO_cm.__exit__(None, None, None)


# ---------------------------------------------------------------------------
# Runner
# ---------------------------------------------------------------------------

_CACHE = {}


def _get_nc(stage="full"):
    if stage not in _CACHE:
        _CACHE[stage] = build(stage)
    return _CACHE[stage]


def run_cores(inputs, stage="full"):
    nc = _get_nc(stage)
    in_maps = [prep_core_inputs(inputs, c) for c in range(N_CORES)]
    res = run_bass_kernel_spmd(nc, in_maps, list(range(N_CORES)))
    return res.results


def kernel(**inputs):
    results = run_cores(inputs, "full")
    out = np.zeros((B, 1, COUT, H, W), np.float32)
    for c in range(N_CORES):
        b, h = c // 2, c % 2
        o = results[c]["out"]                    # [128, 2, 16, 32]
        out[b, 0, :, 16 * h: 16 * h + 16, :] = (
            o.transpose(1, 0, 2, 3).reshape(COUT, 16, 32))
    return out
